# revision 16
# baseline (speedup 1.0000x reference)
"""Trainium2 Bass kernel for nn_Mini_pointgnn_v9 (PointGNN message passing).

Self-contained: host preprocessing (graph bucketing/padding + index builds),
an 8-core SPMD Bass/Tile program, and the kernel() entry that shards, runs via
run_bass_kernel_spmd, and reassembles the full [600000, 20] output.

Design notes
------------
* All segmented reductions (segment_sum / segment_max) become fixed-width
  grouped reductions along the SBUF/PSUM free axis: vertices are bucketed by
  their member count (points per l1 cluster, in-edges per vertex, ...) and
  each vertex's member list is padded to the bucket width K (replicating a
  real member for max-reductions; zero-contribution slots for the sum via a
  mask row folded into the MLP bias path).
* All per-slot MLP math runs feature-major ([feat, slots]) so the 64-wide
  MLPs are single matmuls (weights stationary as lhsT, f32r at 1 cyc/col).
  Gathered table rows arrive slot-major [128 slots, 64] from the indirect
  DMA and are transposed into the PSUM accumulator by TensorE, accumulating
  on top of the rel-position matmul (which also plants the bias and the
  all-ones row used to carry biases through the second MLP layer).
* Vertex feature tables (the gather sources) are bf16, replicated to every
  core with an AllGather between layers; everything else is f32.
* relu commutes with max, so the max-reductions read the second-layer PSUM
  directly and a single relu runs on the reduced [64, nvert] result.
"""

import numpy as np

TRACE = False
LAST = {}

NCORES = 8
TILE = 512
P = 128
CH_TILES = 4          # slot tiles per DMA chunk
GM = 8                # gather blocks (128 rows) per indirect DMA

K_BUCKETS = [4, 8, 12, 16, 20, 24, 28, 32, 40, 48, 64, 96, 128, 256, 512]


def _bucket_of(n):
    for k in K_BUCKETS:
        if n <= k:
            return k
    raise ValueError(f"count {n} exceeds max bucket")


def _bf16(x):
    x = np.asarray(x, np.float32)
    u = x.view(np.uint32)
    rounded = ((u + 0x7FFF + ((u >> 16) & 1)) & 0xFFFF0000).astype(np.uint32)
    return rounded.view(np.float32)


def _deal_groups(keys, order_items):
    from collections import defaultdict

    by_key = defaultdict(list)
    for it in order_items:
        by_key[keys[it]].append(it)
    out = []
    for key in sorted(by_key):
        items = by_key[key]
        percore = [items[c::NCORES] for c in range(NCORES)]
        nmax = max(len(x) for x in percore)
        for x in percore:
            x.extend([-1] * (nmax - len(x)))
        out.append((key, percore))
    return out


def _tile_plan(groups_nv_k):
    plan = []
    for nv, k in groups_nv_k:
        per = max(1, TILE // k)
        left = nv
        while left > 0:
            n = min(per, left)
            plan.append((k, n))
            left -= n
    return plan


def _slots_idx_layout(idx_flat):
    s = idx_flat.reshape(-1, P)
    return np.ascontiguousarray(s.T).astype(np.int32)


class _SlotStruct:
    pass


class _CrossStruct:
    """Vertex deal shared by two slot structures over the same vertex set."""

    def __init__(self, groups, n_vert):
        self.groups = groups
        self.nvc = sum(len(pc[0]) for _, pc in groups)
        pad = (-self.nvc) % TILE
        self.nvc += pad
        self.pad = pad
        self.vorder = []
        for c in range(NCORES):
            vo = []
            for _, pc in groups:
                vo.extend(pc[c])
            vo.extend([-1] * pad)
            self.vorder.append(vo)
        self.vpos = {}
        for c in range(NCORES):
            for i, v in enumerate(self.vorder[c]):
                if v >= 0:
                    self.vpos[v] = (c, i)

    def storage_rows(self, varr):
        varr = np.asarray(varr)
        nb = self.nvc // P
        cp = np.array([self.vpos[int(v)] for v in varr.ravel()], np.int64)
        c, pos = cp[:, 0], cp[:, 1]
        return (c * self.nvc + (pos % P) * nb + pos // P).reshape(varr.shape)

    def build_slots(self, key_axis, members, pad_mode):
        st = _SlotStruct()
        st.nvc = self.nvc
        gnvk = [(len(pc[0]), key[key_axis]) for key, pc in self.groups]
        if self.pad:
            gnvk.append((self.pad, K_BUCKETS[0]))
        st.plan = _tile_plan(gnvk)
        st.n_tiles = len(st.plan)
        st.n_slots = st.n_tiles * TILE
        st.slot_member = np.full((NCORES, st.n_slots), -1, np.int64)
        st.empty_mask = np.ones((NCORES, st.nvc), np.float32)
        for c in range(NCORES):
            vi = 0
            s = 0
            for k, nv in st.plan:
                for j in range(nv):
                    v = self.vorder[c][vi]
                    base = s + j * k
                    if v >= 0 and len(members[v]) > 0:
                        m = members[v]
                        if pad_mode == "replicate":
                            reps = (k + len(m) - 1) // len(m)
                            mm = (m * reps)[:k]
                        else:
                            mm = m + [-1] * (k - len(m))
                        st.slot_member[c, base : base + k] = mm[:k]
                    elif v >= 0:
                        st.empty_mask[c, vi] = 0.0
                    vi += 1
                s += TILE
        st.any_empty = bool((st.empty_mask == 0).any())
        return st


class Prep:
    def __init__(self, params, remission, points, c1, c2, l1_edges, l2_edges,
                 l1_labels, l2_labels):
        self.params = params
        np_pts = points.shape[0]
        n1 = c1.shape[0]
        n2 = c2.shape[0]
        self.n1, self.n2, self.npts = n1, n2, np_pts

        pts_members = [[] for _ in range(n1)]
        for i, lab in enumerate(np.asarray(l1_labels)):
            pts_members[lab].append(i)
        e1 = np.asarray(l1_edges)
        edge_members1 = [[] for _ in range(n1)]
        for ei in range(e1.shape[0]):
            edge_members1[e1[ei, 1]].append(ei)

        counts_p = np.array([max(len(x), 1) for x in pts_members])
        counts_d = np.array([max(len(x), 1) for x in edge_members1])
        keys = {v: (_bucket_of(counts_p[v]), _bucket_of(counts_d[v]))
                for v in range(n1)}
        groups = _deal_groups(keys, list(range(n1)))
        self.l1 = _CrossStruct(groups, n1)

        l1_members2 = [[] for _ in range(n2)]
        for v1, lab2 in enumerate(np.asarray(l2_labels)):
            l1_members2[lab2].append(v1)
        e2 = np.asarray(l2_edges)
        edge_members2 = [[] for _ in range(n2)]
        for ei in range(e2.shape[0]):
            edge_members2[e2[ei, 1]].append(ei)
        counts_m = np.array([max(len(x), 1) for x in l1_members2])
        counts_d2 = np.array([max(len(x), 1) for x in edge_members2])
        keys2 = {v: (_bucket_of(counts_m[v]), _bucket_of(counts_d2[v]))
                 for v in range(n2)}
        groups2 = _deal_groups(keys2, list(range(n2)))
        self.l2 = _CrossStruct(groups2, n2)

        self.L1 = self.l1.build_slots(0, pts_members, "zero")
        self.L2 = self.l1.build_slots(1, edge_members1, "replicate")
        self.L3 = self.l2.build_slots(0, l1_members2, "replicate")
        self.L4 = self.l2.build_slots(1, edge_members2, "replicate")

        pts = np.asarray(points, np.float32)
        rem = np.asarray(remission, np.float32)
        c1 = np.asarray(c1, np.float32)
        c2 = np.asarray(c2, np.float32)
        l1_labels = np.asarray(l1_labels)
        l2_labels = np.asarray(l2_labels)

        self.cores = []
        for c in range(NCORES):
            d = {}
            sm = self.L1.slot_member[c]
            live = sm >= 0
            x = np.zeros((5, sm.shape[0]), np.float32)
            pm = sm[live]
            lab = l1_labels[pm]
            x[0:3, live] = (pts[pm] - c1[lab]).T
            x[3, live] = rem[pm, 0]
            x[4, live] = 1.0
            d["xpts"] = x

            zr1 = NCORES * self.l1.nvc
            zr2 = NCORES * self.l2.nvc
            sm2 = self.L2.slot_member[c]
            live2 = sm2 >= 0
            em = sm2[live2]
            src, dst = e1[em, 0], e1[em, 1]
            idx = np.full(sm2.shape[0], zr1, np.int64)
            idx[live2] = self.l1.storage_rows(src)
            rel = np.zeros((4, sm2.shape[0]), np.float32)
            rel[0:3, live2] = (c1[src] - c1[dst]).T
            rel[3, live2] = 1.0
            d["e1_idx"] = _slots_idx_layout(idx)
            d["e1_rel"] = rel

            sm3 = self.L3.slot_member[c]
            live3 = sm3 >= 0
            v1m = sm3[live3]
            idx3 = np.full(sm3.shape[0], zr1, np.int64)
            idx3[live3] = self.l1.storage_rows(v1m)
            rel3 = np.zeros((4, sm3.shape[0]), np.float32)
            rel3[0:3, live3] = (c1[v1m] - c2[l2_labels[v1m]]).T
            rel3[3, live3] = 1.0
            d["m2l_idx"] = _slots_idx_layout(idx3)
            d["m2l_rel"] = rel3

            sm4 = self.L4.slot_member[c]
            live4 = sm4 >= 0
            em4 = sm4[live4]
            src2 = e2[em4, 0]
            idx4 = np.full(sm4.shape[0], zr2, np.int64)
            idx4[live4] = self.l2.storage_rows(src2)
            d["e2_idx"] = _slots_idx_layout(idx4)

            vo = np.array(self.l1.vorder[c])
            vlive = vo >= 0
            idx5 = np.full(self.l1.nvc, zr2, np.int64)
            idx5[vlive] = self.l2.storage_rows(l2_labels[vo[vlive]])
            rel5 = np.zeros((4, self.l1.nvc), np.float32)
            rel5[0:3, vlive] = (c1[vo[vlive]] - c2[l2_labels[vo[vlive]]]).T
            rel5[3, vlive] = 1.0
            d["l2m_idx"] = _slots_idx_layout(idx5)
            d["l2m_rel"] = rel5

            per = self.npts // NCORES
            p0, p1 = c * per, (c + 1) * per
            sp7 = ((per + TILE - 1) // TILE) * TILE
            self.sp7 = sp7
            idx7 = np.full(sp7, zr1, np.int64)
            lab7 = l1_labels[p0:p1]
            idx7[:per] = self.l1.storage_rows(lab7)
            rel7 = np.zeros((4, sp7), np.float32)
            rel7[0:3, :per] = (pts[p0:p1] - c1[lab7]).T
            rel7[3, :per] = 1.0
            d["pt_idx"] = _slots_idx_layout(idx7)
            d["pt_rel"] = rel7

            self.cores.append(d)

        pr = params
        aug = lambda w, b: np.concatenate(
            [np.asarray(w, np.float32), np.asarray(b, np.float32)[None, :]], 0)
        W = {}
        w1 = aug(pr["ffn"]["d1"]["w"], pr["ffn"]["d1"]["b"])
        w1 = np.concatenate([w1, np.zeros((5, 1), np.float32)], 1)
        w1[4, 64] = 1.0
        W["ffn1"] = w1
        W["ffn2"] = aug(pr["ffn"]["d2"]["w"], pr["ffn"]["d2"]["b"])
        for nm, pe in (("g2", "g2e"), ("g6", "g6e")):
            d1w = np.asarray(pr[pe]["d1"]["w"], np.float32)
            W[nm + "_h"] = aug(d1w[0:64], pr[pe]["d1"]["b"])
            r = np.zeros((4, 65), np.float32)
            r[0:3, 0:64] = d1w[64:67]
            r[3, 64] = 1.0
            W[nm + "_r"] = r
            W[nm + "_2"] = aug(pr[pe]["d2"]["w"], pr[pe]["d2"]["b"])
        for nm in ("g2o", "g6o"):
            W[nm + "_1"] = aug(pr[nm]["d1"]["w"], pr[nm]["d1"]["b"])
            W[nm + "_2"] = aug(pr[nm]["d2"]["w"], pr[nm]["d2"]["b"])
        for nm in ("m2l", "l2m", "fbn"):
            d1w = np.asarray(pr[nm]["d1"]["w"], np.float32)
            W[nm + "_u"] = aug(d1w[0:64], pr[nm]["d1"]["b"])
            r = np.zeros((4, 65), np.float32)
            r[0:3, 0:64] = d1w[64:67]
            r[3, 64] = 1.0
            W[nm + "_r"] = r
            W[nm + "_2"] = aug(pr[nm]["d2"]["w"], pr[nm]["d2"]["b"])
        W["cls"] = aug(pr["cls"]["w"], pr["cls"]["b"])
        self.W = W

    def in_maps(self):
        import ml_dtypes
        bf = ml_dtypes.bfloat16
        maps = []
        for c in range(NCORES):
            d = dict(self.cores[c])
            for k in ("xpts", "e1_rel", "m2l_rel", "l2m_rel", "pt_rel"):
                d[k] = d[k].astype(bf)
            for k, v in self.W.items():
                d["w_" + k] = v.astype(bf)
            maps.append(d)
        return maps


# ======================================================================
# Bass program
# ======================================================================

W_SHAPES = {
    "ffn1": (5, 65), "ffn2": (65, 64),
    "g2_h": (65, 64), "g2_r": (4, 65), "g2_2": (65, 64),
    "g6_h": (65, 64), "g6_r": (4, 65), "g6_2": (65, 64),
    "g2o_1": (65, 64), "g2o_2": (65, 64),
    "g6o_1": (65, 64), "g6o_2": (65, 64),
    "m2l_u": (65, 64), "m2l_r": (4, 65), "m2l_2": (65, 64),
    "l2m_u": (65, 64), "l2m_r": (4, 65), "l2m_2": (65, 64),
    "fbn_u": (65, 64), "fbn_r": (4, 65), "fbn_2": (65, 64),
    "cls": (65, 20),
}


def build_program(cfg):
    import sys
    if "/opt/trn_rl_repo" not in sys.path:
        sys.path.insert(0, "/opt/trn_rl_repo")
    from contextlib import ExitStack
    from concourse import bacc, bass, mybir, tile
    from concourse.masks import make_identity

    f32 = mybir.dt.float32
    f32r = mybir.dt.float32r
    bf16 = mybir.dt.bfloat16
    i32 = mybir.dt.int32
    RELU = mybir.ActivationFunctionType.Relu
    COPY = mybir.ActivationFunctionType.Copy
    AX = mybir.AxisListType.X
    MAX = mybir.AluOpType.max
    ADD = mybir.AluOpType.add
    MULT = mybir.AluOpType.mult

    nvc1, nvc2 = cfg["nvc1"], cfg["nvc2"]
    nb1, nb2 = nvc1 // P, nvc2 // P
    sp7 = cfg["sp7"]

    nc = bacc.Bacc("TRN2", target_bir_lowering=False, debug=False,
                   num_devices=NCORES)
    groups = [list(range(NCORES))]

    ins = {}
    def di(name, shape, dt=f32):
        ins[name] = nc.dram_tensor(name, list(shape), dt, kind="ExternalInput")
        return ins[name]

    xpts = di("xpts", (5, cfg["sp"]), bf16)
    e1_idx = di("e1_idx", (P, cfg["s1"] // P), i32)
    e1_rel = di("e1_rel", (4, cfg["s1"]), bf16)
    m2l_idx = di("m2l_idx", (P, cfg["s3"] // P), i32)
    m2l_rel = di("m2l_rel", (4, cfg["s3"]), bf16)
    e2_idx = di("e2_idx", (P, cfg["s2"] // P), i32)
    l2m_idx = di("l2m_idx", (P, nvc1 // P), i32)
    l2m_rel = di("l2m_rel", (4, nvc1), bf16)
    pt_idx = di("pt_idx", (P, sp7 // P), i32)
    pt_rel = di("pt_rel", (4, sp7), bf16)
    for k, sh in W_SHAPES.items():
        di("w_" + k, sh, bf16)

    out_d = nc.dram_tensor("out", [20, sp7], f32, kind="ExternalOutput")

    # internal DRAM
    def shard_pair(name, nvc):
        sh = nc.dram_tensor(name + "_sh", [nvc, 64], bf16)
        tb = nc.dram_tensor(name + "T", [NCORES * nvc + 8, 64], bf16,
                            addr_space="Shared")
        return sh, tb

    h1_sh, h1T = shard_pair("h1", nvc1)
    u2_sh, u2T = shard_pair("u2", nvc1)
    f3_sh, f3T = shard_pair("f3", nvc2)
    u4_sh, u4T = shard_pair("u4", nvc2)
    h5_sh, h5T = shard_pair("h5", nvc1)
    u6_sh, u6T = shard_pair("u6", nvc1)

    with ExitStack() as ctx:
        tc = ctx.enter_context(tile.TileContext(nc))
        const = ctx.enter_context(tc.tile_pool(name="const", bufs=1))
        big = ctx.enter_context(tc.tile_pool(name="big", bufs=1))
        wp = ctx.enter_context(tc.tile_pool(name="work", bufs=3))
        gp = ctx.enter_context(tc.tile_pool(name="gath", bufs=12))
        wp2 = ctx.enter_context(tc.tile_pool(name="work2", bufs=2))
        pp = ctx.enter_context(tc.tile_pool(name="psum", bufs=2, space="PSUM"))

        # ---- constants ----
        wt = {}
        for k, sh in W_SHAPES.items():
            t = const.tile(list(sh), bf16, tag="w_" + k)
            nc.sync.dma_start(out=t[:], in_=ins["w_" + k][:])
            wt[k] = t
        ident_f = const.tile([P, P], f32, tag="identf")
        make_identity(nc, ident_f[:])
        ident_r = const.tile([P, P], f32r, tag="identr")
        nc.scalar.activation(ident_r[:], ident_f[:], COPY)
        ident_b = const.tile([P, P], bf16, tag="identb")
        nc.scalar.activation(ident_b[:], ident_f[:], COPY)
        zrow = const.tile([8, 64], bf16, tag="zrow")
        nc.vector.memset(zrow[:], 0.0)
        for tb, nvc_ in ((h1T, nvc1), (u2T, nvc1), (f3T, nvc2), (u4T, nvc2),
                         (h5T, nvc1), (u6T, nvc1)):
            nc.sync.dma_start(out=tb[NCORES * nvc_ : NCORES * nvc_ + 8, :],
                              in_=zrow[:])

        R = lambda ap: ap.bitcast(f32r)

        def ones_row(t, n):
            nc.vector.memset(t[64:65, 0:n], 1.0)

        # -------------- helpers --------------
        def slot_stage(idx_d, rel_d, table_d, w_r, w_2, plan, agg, op,
                       n_slots, label):
            """Gather+transpose+rel matmul+relu+mm2, then grouped reduce
            (if op) else relu straight into agg columns ([65, n] big tile)."""
            n_tiles = n_slots // TILE
            ti = 0
            vcol = 0
            for ch0 in range(0, n_tiles, CH_TILES):
                nt = min(CH_TILES, n_tiles - ch0)
                nblk = nt * 4
                ic = wp.tile([P, nblk], i32, tag="idx")
                nc.sync.dma_start(out=ic[:], in_=idx_d[:, ch0 * 4 : ch0 * 4 + nblk])
                rc = wp.tile([4, nt * TILE], bf16, tag="rel")
                nc.sync.dma_start(out=rc[:], in_=rel_d[:, ch0 * TILE : (ch0 + nt) * TILE])
                for t in range(nt):
                    ps = pp.tile([65, TILE], f32, tag="acc")
                    nc.tensor.matmul(
                        out=ps[:], lhsT=w_r[:],
                        rhs=rc[:, t * TILE : (t + 1) * TILE],
                        start=True, stop=False, skip_group_check=True)
                    for m in range(4):
                        b = t * 4 + m
                        gt = gp.tile([P, 64], f32r, tag="g")
                        nc.gpsimd.indirect_dma_start(
                            out=gt[:], out_offset=None, in_=table_d[:],
                            in_offset=bass.IndirectOffsetOnAxis(
                                ap=ic[:, b : b + 1], axis=0))
                        nc.tensor.matmul(
                            out=ps[0:64, m * 128 : (m + 1) * 128].bitcast(f32r),
                            lhsT=gt[:], rhs=ident_r[:],
                            is_transpose=True, start=False, stop=(m == 3),
                            skip_group_check=True)
                    ef1 = wp.tile([65, TILE], bf16, tag="ef1")
                    nc.scalar.activation(ef1[:], ps[:], RELU)
                    ps2 = pp.tile([64, TILE], f32, tag="mm2")
                    nc.tensor.matmul(out=ps2[:], lhsT=w_2[:], rhs=ef1[:],
                                     start=True, stop=True)
                    if op is None:
                        nc.scalar.activation(
                            agg[0:64, ti * TILE : (ti + 1) * TILE], ps2[:], RELU)
                    else:
                        k, nv = plan[ti]
                        nc.vector.tensor_reduce(
                            out=agg[0:64, vcol : vcol + nv],
                            in_=ps2[0:64, 0 : nv * k].rearrange(
                                "p (n k) -> p n k", k=k),
                            axis=AX, op=op)
                        vcol += nv
                    ti += 1

        def gather_max_stage(idx_d, table_d, plan, agg, n_slots):
            """L4: gather + transpose + grouped max (no MLP)."""
            n_tiles = n_slots // TILE
            ti = 0
            vcol = 0
            for ch0 in range(0, n_tiles, CH_TILES):
                nt = min(CH_TILES, n_tiles - ch0)
                nblk = nt * 4
                ic = wp.tile([P, nblk], i32, tag="idx")
                nc.sync.dma_start(out=ic[:], in_=idx_d[:, ch0 * 4 : ch0 * 4 + nblk])
                for t in range(nt):
                    ps = pp.tile([64, TILE], f32, tag="acc")
                    for m in range(4):
                        b = t * 4 + m
                        gt = gp.tile([P, 64], f32r, tag="g")
                        nc.gpsimd.indirect_dma_start(
                            out=gt[:], out_offset=None, in_=table_d[:],
                            in_offset=bass.IndirectOffsetOnAxis(
                                ap=ic[:, b : b + 1], axis=0))
                        nc.tensor.matmul(
                            out=ps[0:64, m * 128 : (m + 1) * 128].bitcast(f32r),
                            lhsT=gt[:], rhs=ident_r[:],
                            is_transpose=True, start=True, stop=True,
                            skip_group_check=True)
                    k, nv = plan[ti]
                    nc.vector.tensor_reduce(
                        out=agg[0:64, vcol : vcol + nv],
                        in_=ps[0:64, 0 : nv * k].rearrange("p (n k) -> p n k", k=k),
                        axis=AX, op=MAX)
                    vcol += nv
                    ti += 1

        def vmm(w, src, n, out_cb):
            """out_cb(chunk_psum, c0, w_chunk) over 512-col chunks of
            matmul(lhsT=w, rhs=aug(src[:, c0:c0+w])); src is f32, cast to
            bf16 per chunk with an all-ones bias row appended."""
            for c0 in range(0, n, TILE):
                w_ = min(TILE, n - c0)
                bc = wp.tile([65, TILE], bf16, tag="bc")
                nc.scalar.activation(bc[0:64, 0:w_], src[0:64, c0 : c0 + w_],
                                     COPY)
                nc.vector.memset(bc[64:65, 0:w_], 1.0)
                psv = pp.tile([64, TILE], f32, tag="mm2")
                nc.tensor.matmul(out=psv[0:64, 0:w_], lhsT=w[:],
                                 rhs=bc[:, 0:w_],
                                 start=True, stop=True)
                out_cb(psv, c0, w_)

        def mk_table(w_key, feat, n, nb, shard, table):
            """Build gather table from feat [65+, n] f32: optional matmul
            (w_key, bias folded via ones row) -> bf16 -> transpose to
            row-major -> shard DRAM -> AllGather into `table`."""
            stg = wp2.tile([P, nb * 64], bf16, tag="stg")
            for c0 in range(0, n, TILE):
                w_ = min(TILE, n - c0)
                uc = wp.tile([64, TILE], bf16, tag="uc")
                if w_key is not None:
                    bc = wp.tile([65, TILE], bf16, tag="bc")
                    nc.scalar.activation(bc[0:64, 0:w_],
                                         feat[0:64, c0 : c0 + w_], COPY)
                    nc.vector.memset(bc[64:65, 0:w_], 1.0)
                    psv = pp.tile([64, TILE], f32, tag="mm2")
                    nc.tensor.matmul(out=psv[0:64, 0:w_], lhsT=wt[w_key][:],
                                     rhs=bc[:, 0:w_], start=True, stop=True)
                    nc.scalar.activation(uc[:, 0:w_], psv[0:64, 0:w_], COPY)
                else:
                    nc.scalar.activation(uc[:, 0:w_],
                                         feat[0:64, c0 : c0 + w_], COPY)
                for bb in range(w_ // 128):
                    b = c0 // 128 + bb
                    pst = pp.tile([P, 64], bf16, tag="tp")
                    nc.tensor.matmul(out=pst[:],
                                     lhsT=uc[:, bb * 128 : (bb + 1) * 128],
                                     rhs=ident_b[0:64, 0:64],
                                     is_transpose=True, start=True, stop=True,
                                     skip_group_check=True)
                    nc.scalar.activation(stg[:, b * 64 : (b + 1) * 64],
                                         pst[:], COPY)
            nc.sync.dma_start(
                out=shard[:].rearrange("(p b) d -> p (b d)", p=P), in_=stg[:])
            nc.gpsimd.collective_compute(
                "AllGather", mybir.AluOpType.bypass, replica_groups=groups,
                ins=[shard[:].opt()],
                outs=[table[0 : NCORES * shard.shape[0], :].opt()])

        # ================= Layer 1: point FFN + seg-sum =================
        f1 = big.tile([65, nvc1], f32, tag="f1")
        ones_row(f1, nvc1)
        n_tiles = cfg["sp"] // TILE
        ti = 0
        vcol = 0
        for ch0 in range(0, n_tiles, CH_TILES):
            nt = min(CH_TILES, n_tiles - ch0)
            xc = wp.tile([5, nt * TILE], bf16, tag="xc")
            nc.sync.dma_start(out=xc[:], in_=xpts[:, ch0 * TILE : (ch0 + nt) * TILE])
            for t in range(nt):
                ps = pp.tile([65, TILE], f32, tag="acc")
                nc.tensor.matmul(out=ps[:], lhsT=wt["ffn1"][:],
                                 rhs=xc[:, t * TILE : (t + 1) * TILE],
                                 start=True, stop=True)
                ef1 = wp.tile([65, TILE], bf16, tag="ef1")
                nc.scalar.activation(ef1[:], ps[:], RELU)
                ps2 = pp.tile([64, TILE], f32, tag="mm2")
                nc.tensor.matmul(out=ps2[:], lhsT=wt["ffn2"][:], rhs=ef1[:],
                                 start=True, stop=True)
                efp = wp.tile([64, TILE], f32, tag="efp")
                nc.scalar.activation(efp[:], ps2[:], RELU)
                k, nv = cfg["plan1"][ti]
                nc.vector.tensor_reduce(
                    out=f1[0:64, vcol : vcol + nv],
                    in_=efp[0:64, 0 : nv * k].rearrange("p (n k) -> p n k", k=k),
                    axis=AX, op=ADD)
                vcol += nv
                ti += 1

        # h1 table
        mk_table("g2_h", f1, nvc1, nb1, h1_sh, h1T)

        # ================= Layer 2: GNN on l1 =================
        agg = big.tile([65, nvc1], f32, tag="agg")
        ones_row(agg, nvc1)
        slot_stage(e1_idx, e1_rel, h1T, wt["g2_r"], wt["g2_2"],
                   cfg["plan2"], agg, MAX, cfg["s1"], "L2")
        nc.scalar.activation(agg[0:64, :], agg[0:64, :], RELU)
        f2 = big.tile([65, nvc1], f32, tag="f2")
        ones_row(f2, nvc1)
        def l2_out(psv, c0, w_):
            o1 = wp.tile([65, TILE], bf16, tag="ef1")
            nc.scalar.activation(o1[0:64, 0:w_], psv[0:64, 0:w_], RELU)
            nc.vector.memset(o1[64:65, 0:w_], 1.0)
            ps2 = pp.tile([64, TILE], f32, tag="acc")
            nc.tensor.matmul(out=ps2[0:64, 0:w_], lhsT=wt["g2o_2"][:],
                             rhs=o1[:, 0:w_], start=True, stop=True)
            nc.vector.scalar_tensor_tensor(
                out=f2[0:64, c0 : c0 + w_], in0=ps2[0:64, 0:w_], scalar=0.0,
                in1=f1[0:64, c0 : c0 + w_], op0=MAX, op1=ADD)
        vmm(wt["g2o_1"], agg, nvc1, l2_out)

        mk_table("m2l_u", f2, nvc1, nb1, u2_sh, u2T)

        # ================= Layer 3: mini-to-large =================
        f3 = big.tile([65, nvc2], f32, tag="f3")
        ones_row(f3, nvc2)
        slot_stage(m2l_idx, m2l_rel, u2T, wt["m2l_r"], wt["m2l_2"],
                   cfg["plan3"], f3, MAX, cfg["s3"], "L3")
        nc.scalar.activation(f3[0:64, :], f3[0:64, :], RELU)
        mk_table(None, f3, nvc2, nb2, f3_sh, f3T)

        # ================= Layer 4: plain GNN on l2 =================
        agg4 = big.tile([64, nvc2], f32, tag="agg4")
        gather_max_stage(e2_idx, f3T, cfg["plan4"], agg4, cfg["s2"])
        f4 = big.tile([65, nvc2], f32, tag="f4")
        ones_row(f4, nvc2)
        nc.vector.tensor_tensor(out=f4[0:64, :], in0=f3[0:64, :],
                                in1=agg4[0:64, :], op=ADD)
        mk_table("l2m_u", f4, nvc2, nb2, u4_sh, u4T)

        # ================= Layer 5: large-to-mini =================
        f5 = big.tile([65, nvc1], f32, tag="f5")
        ones_row(f5, nvc1)
        slot_stage(l2m_idx, l2m_rel, u4T, wt["l2m_r"], wt["l2m_2"],
                   None, f5, None, nvc1, "L5")
        mk_table("g6_h", f5, nvc1, nb1, h5_sh, h5T)
        # s56 = f5 + f2 (in place into f5)
        nc.vector.tensor_tensor(out=f5[0:64, :], in0=f5[0:64, :],
                                in1=f2[0:64, :], op=ADD)

        # ================= Layer 6: GNN on l1 (+skip) =================
        agg6 = big.tile([65, nvc1], f32, tag="agg")
        ones_row(agg6, nvc1)
        slot_stage(e1_idx, e1_rel, h5T, wt["g6_r"], wt["g6_2"],
                   cfg["plan2"], agg6, MAX, cfg["s1"], "L6")
        nc.scalar.activation(agg6[0:64, :], agg6[0:64, :], RELU)
        f6 = big.tile([65, nvc1], f32, tag="f1")
        ones_row(f6, nvc1)
        def l6_out(psv, c0, w_):
            o1 = wp.tile([65, TILE], bf16, tag="ef1")
            nc.scalar.activation(o1[0:64, 0:w_], psv[0:64, 0:w_], RELU)
            nc.vector.memset(o1[64:65, 0:w_], 1.0)
            ps2 = pp.tile([64, TILE], f32, tag="acc")
            nc.tensor.matmul(out=ps2[0:64, 0:w_], lhsT=wt["g6o_2"][:],
                             rhs=o1[:, 0:w_], start=True, stop=True)
            nc.vector.scalar_tensor_tensor(
                out=f6[0:64, c0 : c0 + w_], in0=ps2[0:64, 0:w_], scalar=0.0,
                in1=f5[0:64, c0 : c0 + w_], op0=MAX, op1=ADD)
        vmm(wt["g6o_1"], agg6, nvc1, l6_out)

        mk_table("fbn_u", f6, nvc1, nb1, u6_sh, u6T)

        # ================= Layer 7: per-point MLP + classifier ==========
        n_tiles = sp7 // TILE
        for ch0 in range(0, n_tiles, CH_TILES):
            nt = min(CH_TILES, n_tiles - ch0)
            nblk = nt * 4
            ic = wp.tile([P, nblk], i32, tag="idx")
            nc.sync.dma_start(out=ic[:], in_=pt_idx[:, ch0 * 4 : ch0 * 4 + nblk])
            rc = wp.tile([4, nt * TILE], bf16, tag="rel")
            nc.sync.dma_start(out=rc[:], in_=pt_rel[:, ch0 * TILE : (ch0 + nt) * TILE])
            for t in range(nt):
                ps = pp.tile([65, TILE], f32, tag="acc")
                nc.tensor.matmul(out=ps[:], lhsT=wt["fbn_r"][:],
                                 rhs=rc[:, t * TILE : (t + 1) * TILE],
                                 start=True, stop=False, skip_group_check=True)
                for m in range(4):
                    b = t * 4 + m
                    gt = gp.tile([P, 64], f32r, tag="g")
                    nc.gpsimd.indirect_dma_start(
                        out=gt[:], out_offset=None, in_=u6T[:],
                        in_offset=bass.IndirectOffsetOnAxis(
                            ap=ic[:, b : b + 1], axis=0))
                    nc.tensor.matmul(
                        out=ps[0:64, m * 128 : (m + 1) * 128].bitcast(f32r),
                        lhsT=gt[:], rhs=ident_r[:],
                        is_transpose=True, start=False, stop=(m == 3),
                        skip_group_check=True)
                ef1 = wp.tile([65, TILE], bf16, tag="ef1")
                nc.scalar.activation(ef1[:], ps[:], RELU)
                ps2 = pp.tile([64, TILE], f32, tag="mm2")
                nc.tensor.matmul(out=ps2[:], lhsT=wt["fbn_2"][:], rhs=ef1[:],
                                 start=True, stop=True)
                f7 = wp.tile([65, TILE], bf16, tag="f7")
                nc.scalar.activation(f7[0:64, :], ps2[:], RELU)
                nc.vector.memset(f7[64:65, :], 1.0)
                ps3 = pp.tile([20, TILE], f32, tag="acc")
                nc.tensor.matmul(out=ps3[:], lhsT=wt["cls"][:], rhs=f7[:],
                                 start=True, stop=True)
                ot = wp.tile([20, TILE], f32, tag="ot")
                nc.scalar.activation(ot[:], ps3[:], COPY)
                t0 = (ch0 + t) * TILE
                nc.sync.dma_start(out=out_d[:, t0 : t0 + TILE], in_=ot[:])

    nc.compile()
    return nc


def make_cfg(pr):
    return {
        "sp": pr.L1.n_slots, "s1": pr.L2.n_slots, "s3": pr.L3.n_slots,
        "s2": pr.L4.n_slots, "nvc1": pr.l1.nvc, "nvc2": pr.l2.nvc,
        "sp7": pr.sp7,
        "plan1": pr.L1.plan, "plan2": pr.L2.plan, "plan3": pr.L3.plan,
        "plan4": pr.L4.plan,
    }


def _ensure_ntff_hook():
    """Install antenv.axon_hooks shim so trace=True works under axon.

    Best-effort: retries profiler start after forcing PJRT init; if the
    profiler still refuses, runs untraced instead of crashing.
    """
    import contextlib
    import ctypes
    import sys
    import time as _time
    import types

    if "antenv.axon_hooks" in sys.modules:
        return
    try:
        lib = ctypes.CDLL("/opt/axon/libaxon_pjrt.so")
        lib.axon_start_nrt_profile.argtypes = [
            ctypes.POINTER(ctypes.c_int64), ctypes.c_size_t]
        lib.axon_start_nrt_profile.restype = ctypes.c_int64
        lib.axon_stop_nrt_profile.argtypes = [ctypes.c_char_p]
        lib.axon_stop_nrt_profile.restype = ctypes.c_int64
    except Exception:
        return

    @contextlib.contextmanager
    def _hook(output_dir, device_ids):
        import jax
        import jax.numpy as jnp
        (jnp.zeros((8, 8)) + 1).block_until_ready()
        rc = -1
        for _ in range(10):
            if device_ids:
                ids = (ctypes.c_int64 * len(device_ids))(*device_ids)
                rc = lib.axon_start_nrt_profile(ids, len(device_ids))
            else:
                rc = lib.axon_start_nrt_profile(None, 0)
            if rc == 0:
                break
            _time.sleep(0.5)
        try:
            yield
        finally:
            if rc == 0:
                n = lib.axon_stop_nrt_profile(str(output_dir).encode())

    m = types.ModuleType("antenv.axon_hooks")
    m._hook = _hook
    m.set_axon_ntff_profile_hook = lambda h: setattr(m, "_hook", h)
    m.get_axon_ntff_profile_hook = lambda: m._hook
    sys.modules["antenv.axon_hooks"] = m


def kernel(**inputs):
    import sys
    if "/opt/trn_rl_repo" not in sys.path:
        sys.path.insert(0, "/opt/trn_rl_repo")
    from concourse.bass_utils import run_bass_kernel_spmd
    if TRACE:
        _ensure_ntff_hook()
        import jax.numpy as _jnp
        (_jnp.zeros((8, 8)) + 1).block_until_ready()  # init PJRT for profiler

    pr = Prep(inputs["params"], np.asarray(inputs["remission"]),
              np.asarray(inputs["points"]),
              np.asarray(inputs["l1_cluster_centers"]),
              np.asarray(inputs["l2_cluster_centers"]),
              np.asarray(inputs["l1_edges"]), np.asarray(inputs["l2_edges"]),
              np.asarray(inputs["l1_labels"]), np.asarray(inputs["l2_labels"]))
    nc = build_program(make_cfg(pr))
    res = run_bass_kernel_spmd(nc, pr.in_maps(), core_ids=list(range(NCORES)),
                               trace=TRACE)
    LAST["exec_time_ns"] = res.exec_time_ns
    LAST["res"] = res
    per = pr.npts // NCORES
    outs = []
    for c in range(NCORES):
        o = res.results[c]["out"]  # [20, sp7]
        outs.append(o[:, :per].T)
    return np.ascontiguousarray(np.concatenate(outs, 0).astype(np.float32))


# revision 20
# speedup vs baseline: 1.0004x; 1.0004x over previous
"""Trainium2 Bass kernel for nn_Mini_pointgnn_v9 (PointGNN message passing).

Self-contained: host preprocessing (graph bucketing/padding + index builds),
an 8-core SPMD Bass/Tile program, and the kernel() entry that shards, runs via
run_bass_kernel_spmd, and reassembles the full [600000, 20] output.

Design notes
------------
* All segmented reductions (segment_sum / segment_max) become fixed-width
  grouped reductions along the SBUF/PSUM free axis: vertices are bucketed by
  their member count (points per l1 cluster, in-edges per vertex, ...) and
  each vertex's member list is padded to the bucket width K (replicating a
  real member for max-reductions; zero-contribution slots for the sum via a
  mask row folded into the MLP bias path).
* All per-slot MLP math runs feature-major ([feat, slots]) so the 64-wide
  MLPs are single matmuls (weights stationary as lhsT, f32r at 1 cyc/col).
  Gathered table rows arrive slot-major [128 slots, 64] from the indirect
  DMA and are transposed into the PSUM accumulator by TensorE, accumulating
  on top of the rel-position matmul (which also plants the bias and the
  all-ones row used to carry biases through the second MLP layer).
* Vertex feature tables (the gather sources) are bf16, replicated to every
  core with an AllGather between layers; everything else is f32.
* relu commutes with max, so the max-reductions read the second-layer PSUM
  directly and a single relu runs on the reduced [64, nvert] result.
"""

import numpy as np

TRACE = False
LAST = {}

NCORES = 8
TILE = 512
P = 128
CH_TILES = 3          # slot tiles per DMA chunk
GM = 8                # gather blocks (128 rows) per indirect DMA

K_BUCKETS = [4, 8, 12, 16, 20, 24, 28, 32, 40, 48, 64, 96, 128, 256, 512]


def _bucket_of(n):
    for k in K_BUCKETS:
        if n <= k:
            return k
    raise ValueError(f"count {n} exceeds max bucket")


def _bf16(x):
    x = np.asarray(x, np.float32)
    u = x.view(np.uint32)
    rounded = ((u + 0x7FFF + ((u >> 16) & 1)) & 0xFFFF0000).astype(np.uint32)
    return rounded.view(np.float32)


def _deal_groups(keys, order_items):
    from collections import defaultdict

    by_key = defaultdict(list)
    for it in order_items:
        by_key[keys[it]].append(it)
    out = []
    for key in sorted(by_key):
        items = by_key[key]
        percore = [items[c::NCORES] for c in range(NCORES)]
        nmax = max(len(x) for x in percore)
        for x in percore:
            x.extend([-1] * (nmax - len(x)))
        out.append((key, percore))
    return out


def _tile_plan(groups_nv_k):
    plan = []
    for nv, k in groups_nv_k:
        per = max(1, TILE // k)
        left = nv
        while left > 0:
            n = min(per, left)
            plan.append((k, n))
            left -= n
    return plan


def _slots_idx_layout(idx_flat):
    s = idx_flat.reshape(-1, P)
    return np.ascontiguousarray(s.T).astype(np.int32)


class _SlotStruct:
    pass


class _CrossStruct:
    """Vertex deal shared by two slot structures over the same vertex set."""

    def __init__(self, groups, n_vert):
        self.groups = groups
        self.nvc = sum(len(pc[0]) for _, pc in groups)
        pad = (-self.nvc) % TILE
        self.nvc += pad
        self.pad = pad
        self.vorder = []
        for c in range(NCORES):
            vo = []
            for _, pc in groups:
                vo.extend(pc[c])
            vo.extend([-1] * pad)
            self.vorder.append(vo)
        self.vpos = {}
        for c in range(NCORES):
            for i, v in enumerate(self.vorder[c]):
                if v >= 0:
                    self.vpos[v] = (c, i)

    def storage_rows(self, varr):
        varr = np.asarray(varr)
        nb = self.nvc // P
        cp = np.array([self.vpos[int(v)] for v in varr.ravel()], np.int64)
        c, pos = cp[:, 0], cp[:, 1]
        return (c * self.nvc + (pos % P) * nb + pos // P).reshape(varr.shape)

    def build_slots(self, key_axis, members, pad_mode):
        st = _SlotStruct()
        st.nvc = self.nvc
        gnvk = [(len(pc[0]), key[key_axis]) for key, pc in self.groups]
        if self.pad:
            gnvk.append((self.pad, K_BUCKETS[0]))
        st.plan = _tile_plan(gnvk)
        st.n_tiles = len(st.plan)
        st.n_slots = st.n_tiles * TILE
        st.slot_member = np.full((NCORES, st.n_slots), -1, np.int64)
        st.empty_mask = np.ones((NCORES, st.nvc), np.float32)
        for c in range(NCORES):
            vi = 0
            s = 0
            for k, nv in st.plan:
                for j in range(nv):
                    v = self.vorder[c][vi]
                    base = s + j * k
                    if v >= 0 and len(members[v]) > 0:
                        m = members[v]
                        if pad_mode == "replicate":
                            reps = (k + len(m) - 1) // len(m)
                            mm = (m * reps)[:k]
                        else:
                            mm = m + [-1] * (k - len(m))
                        st.slot_member[c, base : base + k] = mm[:k]
                    elif v >= 0:
                        st.empty_mask[c, vi] = 0.0
                    vi += 1
                s += TILE
        st.any_empty = bool((st.empty_mask == 0).any())
        return st


class Prep:
    def __init__(self, params, remission, points, c1, c2, l1_edges, l2_edges,
                 l1_labels, l2_labels):
        self.params = params
        np_pts = points.shape[0]
        n1 = c1.shape[0]
        n2 = c2.shape[0]
        self.n1, self.n2, self.npts = n1, n2, np_pts

        pts_members = [[] for _ in range(n1)]
        for i, lab in enumerate(np.asarray(l1_labels)):
            pts_members[lab].append(i)
        e1 = np.asarray(l1_edges)
        edge_members1 = [[] for _ in range(n1)]
        for ei in range(e1.shape[0]):
            edge_members1[e1[ei, 1]].append(ei)

        counts_p = np.array([max(len(x), 1) for x in pts_members])
        counts_d = np.array([max(len(x), 1) for x in edge_members1])
        keys = {v: (_bucket_of(counts_p[v]), _bucket_of(counts_d[v]))
                for v in range(n1)}
        groups = _deal_groups(keys, list(range(n1)))
        self.l1 = _CrossStruct(groups, n1)

        l1_members2 = [[] for _ in range(n2)]
        for v1, lab2 in enumerate(np.asarray(l2_labels)):
            l1_members2[lab2].append(v1)
        e2 = np.asarray(l2_edges)
        edge_members2 = [[] for _ in range(n2)]
        for ei in range(e2.shape[0]):
            edge_members2[e2[ei, 1]].append(ei)
        counts_m = np.array([max(len(x), 1) for x in l1_members2])
        counts_d2 = np.array([max(len(x), 1) for x in edge_members2])
        keys2 = {v: (_bucket_of(counts_m[v]), _bucket_of(counts_d2[v]))
                 for v in range(n2)}
        groups2 = _deal_groups(keys2, list(range(n2)))
        self.l2 = _CrossStruct(groups2, n2)

        self.L1 = self.l1.build_slots(0, pts_members, "zero")
        self.L2 = self.l1.build_slots(1, edge_members1, "replicate")
        self.L3 = self.l2.build_slots(0, l1_members2, "replicate")
        self.L4 = self.l2.build_slots(1, edge_members2, "replicate")

        pts = np.asarray(points, np.float32)
        rem = np.asarray(remission, np.float32)
        c1 = np.asarray(c1, np.float32)
        c2 = np.asarray(c2, np.float32)
        l1_labels = np.asarray(l1_labels)
        l2_labels = np.asarray(l2_labels)

        self.cores = []
        for c in range(NCORES):
            d = {}
            sm = self.L1.slot_member[c]
            live = sm >= 0
            x = np.zeros((5, sm.shape[0]), np.float32)
            pm = sm[live]
            lab = l1_labels[pm]
            x[0:3, live] = (pts[pm] - c1[lab]).T
            x[3, live] = rem[pm, 0]
            x[4, live] = 1.0
            d["xpts"] = x

            zr1 = NCORES * self.l1.nvc
            zr2 = NCORES * self.l2.nvc
            sm2 = self.L2.slot_member[c]
            live2 = sm2 >= 0
            em = sm2[live2]
            src, dst = e1[em, 0], e1[em, 1]
            idx = np.full(sm2.shape[0], zr1, np.int64)
            idx[live2] = self.l1.storage_rows(src)
            rel = np.zeros((4, sm2.shape[0]), np.float32)
            rel[0:3, live2] = (c1[src] - c1[dst]).T
            rel[3, live2] = 1.0
            d["e1_idx"] = _slots_idx_layout(idx)
            d["e1_rel"] = rel

            sm3 = self.L3.slot_member[c]
            live3 = sm3 >= 0
            v1m = sm3[live3]
            idx3 = np.full(sm3.shape[0], zr1, np.int64)
            idx3[live3] = self.l1.storage_rows(v1m)
            rel3 = np.zeros((4, sm3.shape[0]), np.float32)
            rel3[0:3, live3] = (c1[v1m] - c2[l2_labels[v1m]]).T
            rel3[3, live3] = 1.0
            d["m2l_idx"] = _slots_idx_layout(idx3)
            d["m2l_rel"] = rel3

            sm4 = self.L4.slot_member[c]
            live4 = sm4 >= 0
            em4 = sm4[live4]
            src2 = e2[em4, 0]
            idx4 = np.full(sm4.shape[0], zr2, np.int64)
            idx4[live4] = self.l2.storage_rows(src2)
            d["e2_idx"] = _slots_idx_layout(idx4)

            vo = np.array(self.l1.vorder[c])
            vlive = vo >= 0
            idx5 = np.full(self.l1.nvc, zr2, np.int64)
            idx5[vlive] = self.l2.storage_rows(l2_labels[vo[vlive]])
            rel5 = np.zeros((4, self.l1.nvc), np.float32)
            rel5[0:3, vlive] = (c1[vo[vlive]] - c2[l2_labels[vo[vlive]]]).T
            rel5[3, vlive] = 1.0
            d["l2m_idx"] = _slots_idx_layout(idx5)
            d["l2m_rel"] = rel5

            per = self.npts // NCORES
            p0, p1 = c * per, (c + 1) * per
            sp7 = ((per + TILE - 1) // TILE) * TILE
            self.sp7 = sp7
            idx7 = np.full(sp7, zr1, np.int64)
            lab7 = l1_labels[p0:p1]
            idx7[:per] = self.l1.storage_rows(lab7)
            rel7 = np.zeros((4, sp7), np.float32)
            rel7[0:3, :per] = (pts[p0:p1] - c1[lab7]).T
            rel7[3, :per] = 1.0
            d["pt_idx"] = _slots_idx_layout(idx7)
            d["pt_rel"] = rel7

            self.cores.append(d)

        pr = params
        aug = lambda w, b: np.concatenate(
            [np.asarray(w, np.float32), np.asarray(b, np.float32)[None, :]], 0)
        W = {}
        w1 = aug(pr["ffn"]["d1"]["w"], pr["ffn"]["d1"]["b"])
        w1 = np.concatenate([w1, np.zeros((5, 1), np.float32)], 1)
        w1[4, 64] = 1.0
        W["ffn1"] = w1
        W["ffn2"] = aug(pr["ffn"]["d2"]["w"], pr["ffn"]["d2"]["b"])
        for nm, pe in (("g2", "g2e"), ("g6", "g6e")):
            d1w = np.asarray(pr[pe]["d1"]["w"], np.float32)
            W[nm + "_h"] = aug(d1w[0:64], pr[pe]["d1"]["b"])
            r = np.zeros((4, 65), np.float32)
            r[0:3, 0:64] = d1w[64:67]
            r[3, 64] = 1.0
            W[nm + "_r"] = r
            W[nm + "_2"] = aug(pr[pe]["d2"]["w"], pr[pe]["d2"]["b"])
        for nm in ("g2o", "g6o"):
            W[nm + "_1"] = aug(pr[nm]["d1"]["w"], pr[nm]["d1"]["b"])
            W[nm + "_2"] = aug(pr[nm]["d2"]["w"], pr[nm]["d2"]["b"])
        for nm in ("m2l", "l2m", "fbn"):
            d1w = np.asarray(pr[nm]["d1"]["w"], np.float32)
            W[nm + "_u"] = aug(d1w[0:64], pr[nm]["d1"]["b"])
            r = np.zeros((4, 65), np.float32)
            r[0:3, 0:64] = d1w[64:67]
            r[3, 64] = 1.0
            W[nm + "_r"] = r
            W[nm + "_2"] = aug(pr[nm]["d2"]["w"], pr[nm]["d2"]["b"])
        W["cls"] = aug(pr["cls"]["w"], pr["cls"]["b"])
        self.W = W

    def in_maps(self):
        import ml_dtypes
        bf = ml_dtypes.bfloat16
        maps = []
        for c in range(NCORES):
            d = dict(self.cores[c])
            f32r_w = {"ffn1", "g2_r", "g6_r", "m2l_r", "l2m_r", "fbn_r"}
            for k, v in self.W.items():
                d["w_" + k] = v if k in f32r_w else v.astype(bf)
            maps.append(d)
        return maps


# ======================================================================
# Bass program
# ======================================================================

W_SHAPES = {
    "ffn1": (5, 65), "ffn2": (65, 64),
    "g2_h": (65, 64), "g2_r": (4, 65), "g2_2": (65, 64),
    "g6_h": (65, 64), "g6_r": (4, 65), "g6_2": (65, 64),
    "g2o_1": (65, 64), "g2o_2": (65, 64),
    "g6o_1": (65, 64), "g6o_2": (65, 64),
    "m2l_u": (65, 64), "m2l_r": (4, 65), "m2l_2": (65, 64),
    "l2m_u": (65, 64), "l2m_r": (4, 65), "l2m_2": (65, 64),
    "fbn_u": (65, 64), "fbn_r": (4, 65), "fbn_2": (65, 64),
    "cls": (65, 20),
}


def build_program(cfg):
    import sys
    if "/opt/trn_rl_repo" not in sys.path:
        sys.path.insert(0, "/opt/trn_rl_repo")
    from contextlib import ExitStack
    from concourse import bacc, bass, mybir, tile
    from concourse.masks import make_identity

    f32 = mybir.dt.float32
    f32r = mybir.dt.float32r
    bf16 = mybir.dt.bfloat16
    i32 = mybir.dt.int32
    RELU = mybir.ActivationFunctionType.Relu
    COPY = mybir.ActivationFunctionType.Copy
    AX = mybir.AxisListType.X
    MAX = mybir.AluOpType.max
    ADD = mybir.AluOpType.add
    MULT = mybir.AluOpType.mult

    nvc1, nvc2 = cfg["nvc1"], cfg["nvc2"]
    nb1, nb2 = nvc1 // P, nvc2 // P
    sp7 = cfg["sp7"]

    nc = bacc.Bacc("TRN2", target_bir_lowering=False, debug=False,
                   num_devices=NCORES)
    groups = [list(range(NCORES))]

    ins = {}
    def di(name, shape, dt=f32):
        ins[name] = nc.dram_tensor(name, list(shape), dt, kind="ExternalInput")
        return ins[name]

    xpts = di("xpts", (5, cfg["sp"]), f32r)
    e1_idx = di("e1_idx", (P, cfg["s1"] // P), i32)
    e1_rel = di("e1_rel", (4, cfg["s1"]), f32r)
    m2l_idx = di("m2l_idx", (P, cfg["s3"] // P), i32)
    m2l_rel = di("m2l_rel", (4, cfg["s3"]), f32r)
    e2_idx = di("e2_idx", (P, cfg["s2"] // P), i32)
    l2m_idx = di("l2m_idx", (P, nvc1 // P), i32)
    l2m_rel = di("l2m_rel", (4, nvc1), f32r)
    pt_idx = di("pt_idx", (P, sp7 // P), i32)
    pt_rel = di("pt_rel", (4, sp7), f32r)
    F32R_W = {"ffn1", "g2_r", "g6_r", "m2l_r", "l2m_r", "fbn_r"}
    for k, sh in W_SHAPES.items():
        di("w_" + k, sh, f32r if k in F32R_W else bf16)

    out_d = nc.dram_tensor("out", [20, sp7], f32, kind="ExternalOutput")

    # internal DRAM
    def shard_pair(name, nvc):
        sh = nc.dram_tensor(name + "_sh", [nvc, 64], bf16)
        tb = nc.dram_tensor(name + "T", [NCORES * nvc + 8, 64], bf16,
                            addr_space="Shared")
        return sh, tb

    h1_sh, h1T = shard_pair("h1", nvc1)
    u2_sh, u2T = shard_pair("u2", nvc1)
    f3_sh, f3T = shard_pair("f3", nvc2)
    u4_sh, u4T = shard_pair("u4", nvc2)
    h5_sh, h5T = shard_pair("h5", nvc1)
    u6_sh, u6T = shard_pair("u6", nvc1)

    with ExitStack() as ctx:
        tc = ctx.enter_context(tile.TileContext(nc))
        const = ctx.enter_context(tc.tile_pool(name="const", bufs=1))
        big = ctx.enter_context(tc.tile_pool(name="big", bufs=1))
        wp = ctx.enter_context(tc.tile_pool(name="work", bufs=3))
        gp = ctx.enter_context(tc.tile_pool(name="gath", bufs=8))
        wp2 = ctx.enter_context(tc.tile_pool(name="work2", bufs=2))
        wp2 = ctx.enter_context(tc.tile_pool(name="work2", bufs=2))
        pp = ctx.enter_context(tc.tile_pool(name="psum", bufs=2, space="PSUM"))

        # ---- constants ----
        wt = {}
        for k, sh in W_SHAPES.items():
            t = const.tile(list(sh), f32r if k in F32R_W else bf16,
                           tag="w_" + k)
            nc.sync.dma_start(out=t[:], in_=ins["w_" + k][:])
            wt[k] = t
        ident_f = const.tile([P, P], f32, tag="identf")
        make_identity(nc, ident_f[:])
        ident_r = const.tile([P, P], f32r, tag="identr")
        nc.scalar.activation(ident_r[:], ident_f[:], COPY)
        ident_b = const.tile([P, P], bf16, tag="identb")
        nc.scalar.activation(ident_b[:], ident_f[:], COPY)
        zrow = const.tile([8, 64], bf16, tag="zrow")
        nc.vector.memset(zrow[:], 0.0)
        for tb, nvc_ in ((h1T, nvc1), (u2T, nvc1), (f3T, nvc2), (u4T, nvc2),
                         (h5T, nvc1), (u6T, nvc1)):
            nc.sync.dma_start(out=tb[NCORES * nvc_ : NCORES * nvc_ + 8, :],
                              in_=zrow[:])

        R = lambda ap: ap.bitcast(f32r)

        def ones_row(t, n):
            nc.vector.memset(t[64:65, 0:n], 1.0)

        # -------------- helpers --------------
        def slot_stage(idx_d, rel_d, table_d, w_r, w_2, plan, agg, op,
                       n_slots, label):
            """Gather+transpose+rel matmul+relu+mm2, then grouped reduce
            (if op) else relu straight into agg columns ([65, n] big tile)."""
            n_tiles = n_slots // TILE
            ti = 0
            vcol = 0
            for ch0 in range(0, n_tiles, CH_TILES):
                nt = min(CH_TILES, n_tiles - ch0)
                nblk = nt * 4
                ic = wp.tile([P, nblk], i32, tag="idx")
                nc.sync.dma_start(out=ic[:], in_=idx_d[:, ch0 * 4 : ch0 * 4 + nblk])
                rc = wp2.tile([4, nt * TILE], f32r, tag="rel")
                nc.sync.dma_start(out=rc[:], in_=rel_d[:, ch0 * TILE : (ch0 + nt) * TILE])
                for t in range(nt):
                    ps = pp.tile([65, TILE], f32, tag="acc")
                    nc.tensor.matmul(
                        out=ps[:], lhsT=w_r[:],
                        rhs=rc[:, t * TILE : (t + 1) * TILE],
                        start=True, stop=False, skip_group_check=True)
                    for m in range(4):
                        b = t * 4 + m
                        gt = gp.tile([P, 64], f32r, tag="g")
                        nc.gpsimd.indirect_dma_start(
                            out=gt[:], out_offset=None, in_=table_d[:],
                            in_offset=bass.IndirectOffsetOnAxis(
                                ap=ic[:, b : b + 1], axis=0))
                        nc.tensor.matmul(
                            out=ps[0:64, m * 128 : (m + 1) * 128].bitcast(f32r),
                            lhsT=gt[:], rhs=ident_r[:],
                            is_transpose=True, start=False, stop=(m == 3),
                            skip_group_check=True)
                    ef1 = wp.tile([65, TILE], bf16, tag="ef1")
                    nc.scalar.activation(ef1[:], ps[:], RELU)
                    ps2 = pp.tile([64, TILE], f32, tag="mm2")
                    nc.tensor.matmul(out=ps2[:], lhsT=w_2[:], rhs=ef1[:],
                                     start=True, stop=True)
                    if op is None:
                        nc.scalar.activation(
                            agg[0:64, ti * TILE : (ti + 1) * TILE], ps2[:], RELU)
                    else:
                        k, nv = plan[ti]
                        nc.vector.tensor_reduce(
                            out=agg[0:64, vcol : vcol + nv],
                            in_=ps2[0:64, 0 : nv * k].rearrange(
                                "p (n k) -> p n k", k=k),
                            axis=AX, op=op)
                        vcol += nv
                    ti += 1

        def gather_max_stage(idx_d, table_d, plan, agg, n_slots):
            """L4: gather + transpose + grouped max (no MLP)."""
            n_tiles = n_slots // TILE
            ti = 0
            vcol = 0
            for ch0 in range(0, n_tiles, CH_TILES):
                nt = min(CH_TILES, n_tiles - ch0)
                nblk = nt * 4
                ic = wp.tile([P, nblk], i32, tag="idx")
                nc.sync.dma_start(out=ic[:], in_=idx_d[:, ch0 * 4 : ch0 * 4 + nblk])
                for t in range(nt):
                    ps = pp.tile([64, TILE], f32, tag="acc")
                    for m in range(4):
                        b = t * 4 + m
                        gt = gp.tile([P, 64], f32r, tag="g")
                        nc.gpsimd.indirect_dma_start(
                            out=gt[:], out_offset=None, in_=table_d[:],
                            in_offset=bass.IndirectOffsetOnAxis(
                                ap=ic[:, b : b + 1], axis=0))
                        nc.tensor.matmul(
                            out=ps[0:64, m * 128 : (m + 1) * 128].bitcast(f32r),
                            lhsT=gt[:], rhs=ident_r[:],
                            is_transpose=True, start=True, stop=True,
                            skip_group_check=True)
                    k, nv = plan[ti]
                    nc.vector.tensor_reduce(
                        out=agg[0:64, vcol : vcol + nv],
                        in_=ps[0:64, 0 : nv * k].rearrange("p (n k) -> p n k", k=k),
                        axis=AX, op=MAX)
                    vcol += nv
                    ti += 1

        def vmm(w, src, n, out_cb):
            """out_cb(chunk_psum, c0, w_chunk) over 512-col chunks of
            matmul(lhsT=w, rhs=aug(src[:, c0:c0+w])); src is f32, cast to
            bf16 per chunk with an all-ones bias row appended."""
            for c0 in range(0, n, TILE):
                w_ = min(TILE, n - c0)
                bc = wp.tile([65, TILE], bf16, tag="bc")
                nc.scalar.activation(bc[0:64, 0:w_], src[0:64, c0 : c0 + w_],
                                     COPY)
                nc.vector.memset(bc[64:65, 0:w_], 1.0)
                psv = pp.tile([64, TILE], f32, tag="mm2")
                nc.tensor.matmul(out=psv[0:64, 0:w_], lhsT=w[:],
                                 rhs=bc[:, 0:w_],
                                 start=True, stop=True)
                out_cb(psv, c0, w_)

        def mk_table(w_key, feat, n, nb, shard, table):
            """Build gather table from feat [65+, n] f32: optional matmul
            (w_key, bias folded via ones row) -> bf16 -> transpose to
            row-major -> shard DRAM -> AllGather into `table`."""
            stg = wp2.tile([P, nb * 64], bf16, tag="stg")
            for c0 in range(0, n, TILE):
                w_ = min(TILE, n - c0)
                uc = wp.tile([64, TILE], bf16, tag="uc")
                if w_key is not None:
                    bc = wp.tile([65, TILE], bf16, tag="bc")
                    nc.scalar.activation(bc[0:64, 0:w_],
                                         feat[0:64, c0 : c0 + w_], COPY)
                    nc.vector.memset(bc[64:65, 0:w_], 1.0)
                    psv = pp.tile([64, TILE], f32, tag="mm2")
                    nc.tensor.matmul(out=psv[0:64, 0:w_], lhsT=wt[w_key][:],
                                     rhs=bc[:, 0:w_], start=True, stop=True)
                    nc.scalar.activation(uc[:, 0:w_], psv[0:64, 0:w_], COPY)
                else:
                    nc.scalar.activation(uc[:, 0:w_],
                                         feat[0:64, c0 : c0 + w_], COPY)
                for bb in range(w_ // 128):
                    b = c0 // 128 + bb
                    pst = pp.tile([P, 64], bf16, tag="tp")
                    nc.tensor.matmul(out=pst[:],
                                     lhsT=uc[:, bb * 128 : (bb + 1) * 128],
                                     rhs=ident_b[0:64, 0:64],
                                     is_transpose=True, start=True, stop=True,
                                     skip_group_check=True)
                    nc.scalar.activation(stg[:, b * 64 : (b + 1) * 64],
                                         pst[:], COPY)
            nc.sync.dma_start(
                out=shard[:].rearrange("(p b) d -> p (b d)", p=P), in_=stg[:])
            nc.gpsimd.collective_compute(
                "AllGather", mybir.AluOpType.bypass, replica_groups=groups,
                ins=[shard[:].opt()],
                outs=[table[0 : NCORES * shard.shape[0], :].opt()])

        # ================= Layer 1: point FFN + seg-sum =================
        f1 = big.tile([65, nvc1], f32, tag="f1")
        ones_row(f1, nvc1)
        n_tiles = cfg["sp"] // TILE
        ti = 0
        vcol = 0
        for ch0 in range(0, n_tiles, CH_TILES):
            nt = min(CH_TILES, n_tiles - ch0)
            xc = wp2.tile([5, nt * TILE], f32r, tag="xc")
            nc.sync.dma_start(out=xc[:], in_=xpts[:, ch0 * TILE : (ch0 + nt) * TILE])
            for t in range(nt):
                ps = pp.tile([65, TILE], f32, tag="acc")
                nc.tensor.matmul(out=ps[:], lhsT=wt["ffn1"][:],
                                 rhs=xc[:, t * TILE : (t + 1) * TILE],
                                 start=True, stop=True)
                ef1 = wp.tile([65, TILE], bf16, tag="ef1")
                nc.scalar.activation(ef1[:], ps[:], RELU)
                ps2 = pp.tile([64, TILE], f32, tag="mm2")
                nc.tensor.matmul(out=ps2[:], lhsT=wt["ffn2"][:], rhs=ef1[:],
                                 start=True, stop=True)
                efp = wp.tile([64, TILE], f32, tag="efp")
                nc.scalar.activation(efp[:], ps2[:], RELU)
                k, nv = cfg["plan1"][ti]
                nc.vector.tensor_reduce(
                    out=f1[0:64, vcol : vcol + nv],
                    in_=efp[0:64, 0 : nv * k].rearrange("p (n k) -> p n k", k=k),
                    axis=AX, op=ADD)
                vcol += nv
                ti += 1

        # h1 table
        mk_table("g2_h", f1, nvc1, nb1, h1_sh, h1T)

        # ================= Layer 2: GNN on l1 =================
        agg = big.tile([65, nvc1], f32, tag="agg")
        ones_row(agg, nvc1)
        slot_stage(e1_idx, e1_rel, h1T, wt["g2_r"], wt["g2_2"],
                   cfg["plan2"], agg, MAX, cfg["s1"], "L2")
        nc.scalar.activation(agg[0:64, :], agg[0:64, :], RELU)
        f2 = big.tile([65, nvc1], f32, tag="f2")
        ones_row(f2, nvc1)
        def l2_out(psv, c0, w_):
            o1 = wp.tile([65, TILE], bf16, tag="ef1")
            nc.scalar.activation(o1[0:64, 0:w_], psv[0:64, 0:w_], RELU)
            nc.vector.memset(o1[64:65, 0:w_], 1.0)
            ps2 = pp.tile([64, TILE], f32, tag="acc")
            nc.tensor.matmul(out=ps2[0:64, 0:w_], lhsT=wt["g2o_2"][:],
                             rhs=o1[:, 0:w_], start=True, stop=True)
            nc.vector.scalar_tensor_tensor(
                out=f2[0:64, c0 : c0 + w_], in0=ps2[0:64, 0:w_], scalar=0.0,
                in1=f1[0:64, c0 : c0 + w_], op0=MAX, op1=ADD)
        vmm(wt["g2o_1"], agg, nvc1, l2_out)

        mk_table("m2l_u", f2, nvc1, nb1, u2_sh, u2T)

        # ================= Layer 3: mini-to-large =================
        f3 = big.tile([65, nvc2], f32, tag="f3")
        ones_row(f3, nvc2)
        slot_stage(m2l_idx, m2l_rel, u2T, wt["m2l_r"], wt["m2l_2"],
                   cfg["plan3"], f3, MAX, cfg["s3"], "L3")
        nc.scalar.activation(f3[0:64, :], f3[0:64, :], RELU)
        mk_table(None, f3, nvc2, nb2, f3_sh, f3T)

        # ================= Layer 4: plain GNN on l2 =================
        agg4 = big.tile([64, nvc2], f32, tag="agg4")
        gather_max_stage(e2_idx, f3T, cfg["plan4"], agg4, cfg["s2"])
        f4 = big.tile([65, nvc2], f32, tag="f4")
        ones_row(f4, nvc2)
        nc.vector.tensor_tensor(out=f4[0:64, :], in0=f3[0:64, :],
                                in1=agg4[0:64, :], op=ADD)
        mk_table("l2m_u", f4, nvc2, nb2, u4_sh, u4T)

        # ================= Layer 5: large-to-mini =================
        f5 = big.tile([65, nvc1], f32, tag="f5")
        ones_row(f5, nvc1)
        slot_stage(l2m_idx, l2m_rel, u4T, wt["l2m_r"], wt["l2m_2"],
                   None, f5, None, nvc1, "L5")
        mk_table("g6_h", f5, nvc1, nb1, h5_sh, h5T)
        # s56 = f5 + f2 (in place into f5)
        nc.vector.tensor_tensor(out=f5[0:64, :], in0=f5[0:64, :],
                                in1=f2[0:64, :], op=ADD)

        # ================= Layer 6: GNN on l1 (+skip) =================
        agg6 = big.tile([65, nvc1], f32, tag="agg")
        ones_row(agg6, nvc1)
        slot_stage(e1_idx, e1_rel, h5T, wt["g6_r"], wt["g6_2"],
                   cfg["plan2"], agg6, MAX, cfg["s1"], "L6")
        nc.scalar.activation(agg6[0:64, :], agg6[0:64, :], RELU)
        f6 = big.tile([65, nvc1], f32, tag="f1")
        ones_row(f6, nvc1)
        def l6_out(psv, c0, w_):
            o1 = wp.tile([65, TILE], bf16, tag="ef1")
            nc.scalar.activation(o1[0:64, 0:w_], psv[0:64, 0:w_], RELU)
            nc.vector.memset(o1[64:65, 0:w_], 1.0)
            ps2 = pp.tile([64, TILE], f32, tag="acc")
            nc.tensor.matmul(out=ps2[0:64, 0:w_], lhsT=wt["g6o_2"][:],
                             rhs=o1[:, 0:w_], start=True, stop=True)
            nc.vector.scalar_tensor_tensor(
                out=f6[0:64, c0 : c0 + w_], in0=ps2[0:64, 0:w_], scalar=0.0,
                in1=f5[0:64, c0 : c0 + w_], op0=MAX, op1=ADD)
        vmm(wt["g6o_1"], agg6, nvc1, l6_out)

        mk_table("fbn_u", f6, nvc1, nb1, u6_sh, u6T)

        # ================= Layer 7: per-point MLP + classifier ==========
        n_tiles = sp7 // TILE
        for ch0 in range(0, n_tiles, CH_TILES):
            nt = min(CH_TILES, n_tiles - ch0)
            nblk = nt * 4
            ic = wp.tile([P, nblk], i32, tag="idx")
            nc.sync.dma_start(out=ic[:], in_=pt_idx[:, ch0 * 4 : ch0 * 4 + nblk])
            rc = wp2.tile([4, nt * TILE], f32r, tag="rel")
            nc.sync.dma_start(out=rc[:], in_=pt_rel[:, ch0 * TILE : (ch0 + nt) * TILE])
            for t in range(nt):
                ps = pp.tile([65, TILE], f32, tag="acc")
                nc.tensor.matmul(out=ps[:], lhsT=wt["fbn_r"][:],
                                 rhs=rc[:, t * TILE : (t + 1) * TILE],
                                 start=True, stop=False, skip_group_check=True)
                for m in range(4):
                    b = t * 4 + m
                    gt = gp.tile([P, 64], f32r, tag="g")
                    nc.gpsimd.indirect_dma_start(
                        out=gt[:], out_offset=None, in_=u6T[:],
                        in_offset=bass.IndirectOffsetOnAxis(
                            ap=ic[:, b : b + 1], axis=0))
                    nc.tensor.matmul(
                        out=ps[0:64, m * 128 : (m + 1) * 128].bitcast(f32r),
                        lhsT=gt[:], rhs=ident_r[:],
                        is_transpose=True, start=False, stop=(m == 3),
                        skip_group_check=True)
                ef1 = wp.tile([65, TILE], bf16, tag="ef1")
                nc.scalar.activation(ef1[:], ps[:], RELU)
                ps2 = pp.tile([64, TILE], f32, tag="mm2")
                nc.tensor.matmul(out=ps2[:], lhsT=wt["fbn_2"][:], rhs=ef1[:],
                                 start=True, stop=True)
                f7 = wp.tile([65, TILE], bf16, tag="f7")
                nc.scalar.activation(f7[0:64, :], ps2[:], RELU)
                nc.vector.memset(f7[64:65, :], 1.0)
                ps3 = pp.tile([20, TILE], f32, tag="acc")
                nc.tensor.matmul(out=ps3[:], lhsT=wt["cls"][:], rhs=f7[:],
                                 start=True, stop=True)
                ot = wp2.tile([20, TILE], f32, tag="ot")
                nc.scalar.activation(ot[:], ps3[:], COPY)
                t0 = (ch0 + t) * TILE
                nc.sync.dma_start(out=out_d[:, t0 : t0 + TILE], in_=ot[:])

    nc.compile()
    return nc


def make_cfg(pr):
    return {
        "sp": pr.L1.n_slots, "s1": pr.L2.n_slots, "s3": pr.L3.n_slots,
        "s2": pr.L4.n_slots, "nvc1": pr.l1.nvc, "nvc2": pr.l2.nvc,
        "sp7": pr.sp7,
        "plan1": pr.L1.plan, "plan2": pr.L2.plan, "plan3": pr.L3.plan,
        "plan4": pr.L4.plan,
    }


def _ensure_ntff_hook():
    """Install antenv.axon_hooks shim so trace=True works under axon.

    Best-effort: retries profiler start after forcing PJRT init; if the
    profiler still refuses, runs untraced instead of crashing.
    """
    import contextlib
    import ctypes
    import sys
    import time as _time
    import types

    if "antenv.axon_hooks" in sys.modules:
        return
    try:
        lib = ctypes.CDLL("/opt/axon/libaxon_pjrt.so")
        lib.axon_start_nrt_profile.argtypes = [
            ctypes.POINTER(ctypes.c_int64), ctypes.c_size_t]
        lib.axon_start_nrt_profile.restype = ctypes.c_int64
        lib.axon_stop_nrt_profile.argtypes = [ctypes.c_char_p]
        lib.axon_stop_nrt_profile.restype = ctypes.c_int64
    except Exception:
        return

    @contextlib.contextmanager
    def _hook(output_dir, device_ids):
        import jax
        import jax.numpy as jnp
        (jnp.zeros((8, 8)) + 1).block_until_ready()
        rc = -1
        for _ in range(10):
            if device_ids:
                ids = (ctypes.c_int64 * len(device_ids))(*device_ids)
                rc = lib.axon_start_nrt_profile(ids, len(device_ids))
            else:
                rc = lib.axon_start_nrt_profile(None, 0)
            if rc == 0:
                break
            _time.sleep(0.5)
        try:
            yield
        finally:
            if rc == 0:
                n = lib.axon_stop_nrt_profile(str(output_dir).encode())

    m = types.ModuleType("antenv.axon_hooks")
    m._hook = _hook
    m.set_axon_ntff_profile_hook = lambda h: setattr(m, "_hook", h)
    m.get_axon_ntff_profile_hook = lambda: m._hook
    sys.modules["antenv.axon_hooks"] = m


def kernel(**inputs):
    import sys
    if "/opt/trn_rl_repo" not in sys.path:
        sys.path.insert(0, "/opt/trn_rl_repo")
    from concourse.bass_utils import run_bass_kernel_spmd
    if TRACE:
        _ensure_ntff_hook()
        import jax.numpy as _jnp
        (_jnp.zeros((8, 8)) + 1).block_until_ready()  # init PJRT for profiler

    pr = Prep(inputs["params"], np.asarray(inputs["remission"]),
              np.asarray(inputs["points"]),
              np.asarray(inputs["l1_cluster_centers"]),
              np.asarray(inputs["l2_cluster_centers"]),
              np.asarray(inputs["l1_edges"]), np.asarray(inputs["l2_edges"]),
              np.asarray(inputs["l1_labels"]), np.asarray(inputs["l2_labels"]))
    nc = build_program(make_cfg(pr))
    res = run_bass_kernel_spmd(nc, pr.in_maps(), core_ids=list(range(NCORES)),
                               trace=TRACE)
    LAST["exec_time_ns"] = res.exec_time_ns
    LAST["res"] = res
    per = pr.npts // NCORES
    outs = []
    for c in range(NCORES):
        o = res.results[c]["out"]  # [20, sp7]
        outs.append(o[:, :per].T)
    return np.ascontiguousarray(np.concatenate(outs, 0).astype(np.float32))
